# revision 1
# baseline (speedup 1.0000x reference)
"""Bass/Trainium2 LSTM layer kernel, 8-core SPMD.

Problem: LSTM with B=32, T=512, D=512 (input dim), U=1024 (units).
  x_gates = data @ Wx + b                      # [B, T, 4U]  (phase 1)
  per step: gates = x_gates[:, t] + h @ Wh     # recurrence  (phase 2)
            i, f, g, o = split(gates); c = sig(f)*c + sig(i)*tanh(g)
            h = sig(o)*tanh(c);  output h for every step

Sharding: tensor-parallel over the 1024 units; core c owns units
[128c, 128c+128). Everything runs transposed (units on partitions, batch on
the free dim): the recurrent matmul uses Wh tiles as the stationary operand
and h^T as the moving operand, producing gates^T [128 units, 32 batch] with
no transposes anywhere.

Cross-core h exchange per step uses remote_dma_broadcast (SBUF->SBUF fabric
DMA): each core broadcasts its [128, 32] bf16 h-slice into slot pid*32 of a
[128, 256] gather buffer on all 8 cores, double-buffered across steps, with
remote-semaphore arrival counting.

Phase 1 is interleaved into the recurrence's dead time (the broadcast
flight), which also keeps the PE's HAM clock-gate warm.

All matmuls are bf16 with fp32 PSUM accumulation; c and the h output stay
fp32.
"""

import sys

if "/opt/trn_rl_repo" not in sys.path:
    sys.path.insert(0, "/opt/trn_rl_repo")

import ml_dtypes
import numpy as np

B, T, D, U = 32, 512, 512, 1024
N_CORES = 8
UC = U // N_CORES  # units per core = 128
NT_P1 = 32  # phase-1 moving tiles (16 timesteps * 32 batch each)
P1_COLS = (T * B) // NT_P1  # 512
XG_PITCH = T * 128  # xgT free size per partition
P1_PRO_N = 3  # phase-1 tiles computed in the prologue
P1_PRO_G = 4 * P1_PRO_N  # prologue (n, j) groups
P1_DMA_PRO_N = 6  # staging tiles DMA'd in the prologue (= N_STAG)

_compiled = None
LAST_EXEC_NS = None
GAP_JUNK = True
CAST_ON_ACT = True
DVE_DIRECT_SEND = True
TRACE = False
TRACE_DIR = None


def _build():
    import concourse.bacc as bacc
    import concourse.bass as bass
    import concourse.mybir as mybir

    dt = mybir.dt
    AF = mybir.ActivationFunctionType

    T_ = T
    NT_P1_ = (T_ * B) // P1_COLS
    XG_PITCH_ = T_ * 128
    P1_PRO_N_ = min(P1_PRO_N, NT_P1_)
    P1_PRO_G_ = 4 * P1_PRO_N_
    P1_DMA_PRO_N_ = NT_P1_ if P1_PRO_N_ >= NT_P1_ else min(P1_DMA_PRO_N, NT_P1_)

    nc = bacc.Bacc("TRN2", target_bir_lowering=False, detect_race_conditions=False)

    # ---- DRAM I/O ----
    whT_d = nc.dram_tensor("whT", [128, 8 * 4 * 128], dt.bfloat16, kind="ExternalInput")
    wxT_d = nc.dram_tensor("wxT", [128, 4 * 4 * 128], dt.bfloat16, kind="ExternalInput")
    ident_d = nc.dram_tensor("ident", [128, 128], dt.bfloat16, kind="ExternalInput")
    bT_d = nc.dram_tensor("bT", [128, 4], dt.float32, kind="ExternalInput")
    cfg_d = nc.dram_tensor("cfg", [1, 1], dt.uint32, kind="ExternalInput")
    # data^T, k-chunked: dataT[k][p][t*32+b] = data[b, t, 128k+p]
    dataT_d = nc.dram_tensor("dataT", [4, 128, T_ * B], dt.bfloat16, kind="ExternalInput")
    # output: h^T per step [t][unit][batch] fp32
    out_d = nc.dram_tensor("out_hT", [T_, 128, B], dt.bfloat16, kind="ExternalOutput")

    # ---- SBUF ----
    whT = nc.alloc_sbuf_tensor("whT_sb", [128, 8 * 4 * 128], dt.bfloat16)
    wxT = nc.alloc_sbuf_tensor("wxT_sb", [128, 4 * 4 * 128], dt.bfloat16)
    ident = nc.alloc_sbuf_tensor("ident_sb", [128, 128], dt.bfloat16)
    bT = nc.alloc_sbuf_tensor("bT_sb", [128, 4], dt.float32)
    cfg_sb = nc.alloc_sbuf_tensor("cfg_sb", [1, 1], dt.uint32)
    xgT = nc.alloc_sbuf_tensor("xgT_sb", [128, XG_PITCH_], dt.bfloat16)
    N_STAG = 6
    stag = [
        nc.alloc_sbuf_tensor(f"stag{i}", [128, 4 * P1_COLS], dt.bfloat16)
        for i in range(N_STAG)
    ]
    hT = [nc.alloc_sbuf_tensor(f"hT{i}", [128, 256], dt.bfloat16) for i in range(2)]
    h_send = [nc.alloc_sbuf_tensor(f"hs{i}", [128, B], dt.bfloat16) for i in range(2)]
    h_f32 = [nc.alloc_sbuf_tensor(f"hf{i}", [128, B], dt.float32) for i in range(2)]
    sig_sb = [nc.alloc_sbuf_tensor(f"sig{i}", [128, 96], dt.float32) for i in range(2)]
    g_sb = [nc.alloc_sbuf_tensor(f"g{i}", [128, 32], dt.float32) for i in range(2)]
    tc_sb = [nc.alloc_sbuf_tensor(f"tc{i}", [128, 32], dt.float32) for i in range(2)]
    ig_sb = nc.alloc_sbuf_tensor("ig", [128, 32], dt.float32)
    c_sb = nc.alloc_sbuf_tensor("c", [128, 32], dt.float32)

    # ---- PSUM ----
    ps1 = [
        nc.alloc_psum_tensor(f"ps1_{j}", [128, P1_COLS], dt.float32) for j in range(4)
    ]
    ps = [nc.alloc_psum_tensor(f"ps{i}", [128, 128], dt.float32) for i in range(2)]
    ps_junk = nc.alloc_psum_tensor("ps_junk", [128, 128], dt.float32)

    # ---- semaphores ----
    ld_sem = nc.alloc_semaphore("ld_sem")
    p1d_sems = [nc.alloc_semaphore(f"p1d_sem{i}") for i in range(6)]
    p1m_sem = nc.alloc_semaphore("p1m_sem")
    p1c_sem = nc.alloc_semaphore("p1c_sem")
    rsems = [nc.alloc_semaphore(f"rsem{i}") for i in range(2)]
    lsems = [nc.alloc_semaphore(f"lsem{i}") for i in range(2)]
    blsem = nc.alloc_semaphore("blsem")
    prep_sem = nc.alloc_semaphore("prep_sem")
    bar_sem = nc.alloc_semaphore("bar_sem")
    mm_sem = nc.alloc_semaphore("mm_sem")
    actsig_sem = nc.alloc_semaphore("actsig_sem")
    acts_sem = nc.alloc_semaphore("acts_sem")
    acttc_sem = nc.alloc_semaphore("acttc_sem")
    dvec_sem = nc.alloc_semaphore("dvec_sem")
    dveh_sem = nc.alloc_semaphore("dveh_sem")
    dcast_sem = nc.alloc_semaphore("dcast_sem")
    odma_sems = [nc.alloc_semaphore(f"odma_sem{i}") for i in range(2)]
    init_sem = nc.alloc_semaphore("init_sem")

    RD = [(0, k) for k in range(N_CORES)]
    NB = T_ - 1  # h broadcasts (none after the last step)

    def xg_dst(n, j):
        # strided xgT dest for phase-1 tile n, gate block j:
        # cols 2048n + 128*t_loc + 32j + b
        return bass.AP(
            xgT, n * 16 * 128 + 32 * j, [[XG_PITCH_, 128], [128, 16], [1, 32]]
        )

    # phase-1 (n, j) group schedule: groups for n < P1_PRO_N run in the PE
    # prologue; the rest are interleaved one per scan step starting at step 1.
    def p1_group_at_step(t):
        g = P1_PRO_G_ + t
        if g >= 4 * NT_P1_:
            return None
        return (g // 4, g % 4)

    def p1_group_index(n, j):
        return 4 * n + j

    with nc.Block() as blk:

        # ---------------- SYNC engine: all HWDGE DMA traffic ----------------
        @blk.sync
        def _(s):
            s.dma_start(whT[:], whT_d[:]).then_inc(ld_sem, 16)
            s.dma_start(wxT[:], wxT_d[:]).then_inc(ld_sem, 16)
            s.dma_start(ident[:], ident_d[:]).then_inc(ld_sem, 16)
            s.dma_start(bT[:], bT_d[:]).then_inc(ld_sem, 16)
            s.dma_start(cfg_sb[:], cfg_d[:]).then_inc(ld_sem, 16)

            def stag_dma(n):
                if n >= N_STAG:
                    # stag[n%N_STAG] overwritten: PE must have finished tile
                    # n-N_STAG (runs ~8 steps before this wait is reached, so
                    # it never actually blocks the out-DMA stream)
                    s.wait_ge(p1m_sem, 4 * (n - N_STAG) + 4)
                for k in range(4):
                    s.dma_start(
                        stag[n % N_STAG][:, k * P1_COLS : (k + 1) * P1_COLS],
                        dataT_d[k, :, n * P1_COLS : (n + 1) * P1_COLS],
                    ).then_inc(p1d_sems[n % N_STAG], 16)

            for n in range(P1_DMA_PRO_N_):
                stag_dma(n)
            for t in range(T_):
                s.wait_ge(dveh_sem, t + 1)
                if t < T_ - 1:
                    # hold the 16KB HBM write until this step's h-broadcast
                    # has landed, so the SDMA engines are clear during the
                    # latency-critical flight; the write then overlaps the
                    # next step's compute instead.
                    s.wait_ge(rsems[(t + 1) % 2], 16 * (t // 2 + 1))
                s.dma_start(out_d[t], h_send[t % 2][:]).then_inc(odma_sems[t % 2], 16)
                # one staging tile every 4 steps, staying ~20 steps ahead
                if t % 4 == 0:
                    n = t // 4 + P1_DMA_PRO_N_
                    if n < NT_P1_:
                        stag_dma(n)
            s.wait_ge(odma_sems[0], 16 * ((T_ + 1) // 2))
            s.wait_ge(odma_sems[1], 16 * (T_ // 2))

        # ---------------- GPSIMD: init, barrier, broadcast plumbing ----------
        @blk.gpsimd
        def _(g):
            g.memset(c_sb[:], 1.0).then_inc(init_sem, 1)  # c0 = ones
            g.memset(hT[0][:], 0).then_inc(init_sem, 1)  # h_{-1} = 0
            g.memset(hT[1][:], 0).then_inc(init_sem, 1)
            g.wait_ge(ld_sem, 80)
            pid_r = g.alloc_register("my_pid")
            g.reg_load(pid_r, cfg_sb[0:1, 0:1])
            pid = g.snap(pid_r, donate=True, min_val=0, max_val=N_CORES - 1)
            # init barrier: nobody broadcasts h before everyone zeroed hT bufs
            g.remote_sem_update_broadcast(bar_sem, blsem, rdests=RD).then_inc(
                prep_sem, 1
            )
            g.wait_ge(prep_sem, 1)
            g.trigger_dma(1)
            g.wait_ge(bar_sem, 16)

            def prep(t):
                g.remote_dma_broadcast(
                    hT[(t + 1) % 2][:, bass.ds(pid * 32, 32)],
                    h_send[t % 2][:],
                    rsems[(t + 1) % 2],
                    lsems[t % 2],
                    rdests=RD,
                ).then_inc(prep_sem, 1)

            for t in range(min(2, NB)):
                prep(t)
            for t in range(NB):
                g.wait_ge(prep_sem, 2 + t)
                g.wait_ge(dcast_sem, t + 1)
                g.trigger_dma(1)
                if t + 2 < NB:
                    prep(t + 2)
            g.wait_ge(lsems[0], 16 * ((NB + 1) // 2))
            g.wait_ge(lsems[1], 16 * (NB // 2))  # all sends drained before exit

        # ---------------- TENSOR engine ----------------
        @blk.tensor
        def _(te):
            te.wait_ge(ld_sem, 80)
            te.wait_ge(init_sem, 3)

            def p1_mms(n, j):
                # 4 accumulating matmuls into ps1[j] for tile n
                if n >= 1:
                    # ps1[j] free once the previous group in bank j is copied
                    te.wait_ge(p1c_sem, p1_group_index(n - 1, j) + 1)
                for k in range(4):
                    te.matmul(
                        ps1[j][:],
                        wxT[:, (k * 4 + j) * 128 : (k * 4 + j) * 128 + 128],
                        stag[n % N_STAG][:, k * P1_COLS : (k + 1) * P1_COLS],
                        start=(k == 0),
                        stop=(k == 3),
                    ).then_maybe_inc((p1m_sem, 1) if k == 3 else None)

            def junk_mm():
                te.matmul(
                    ps_junk[:, 0:1],
                    ident[:, 0:128],
                    ident[:, 0:1],
                    start=True,
                    stop=True,
                    skip_group_check=True,
                )

            def fill_mm():
                # N=128 junk matmul: dense PE activity in the broadcast gap
                # keeps the HAM clock gate at 8/8 (no 3.4us idle window)
                te.matmul(
                    ps_junk[:],
                    ident[:],
                    ident[:],
                    start=True,
                    stop=True,
                    skip_group_check=True,
                )

            # prologue: first P1_PRO_N_ tiles
            for n in range(P1_PRO_N_):
                te.wait_ge(p1d_sems[n % N_STAG], 64 * (n // N_STAG + 1))
                for j in range(4):
                    p1_mms(n, j)

            # scan
            for t in range(T_):
                # xg load for step t: ps[t%2] freed by ACT(t-2)'s tanh
                if t >= 2:
                    te.wait_ge(acts_sem, t - 1)
                te.wait_ge(p1c_sem, p1_group_index(t // 16, 3) + 1)
                te.matmul(
                    ps[t % 2][:],
                    ident[:],
                    xgT[:, 128 * t : 128 * t + 128],
                    start=True,
                    stop=False,
                    skip_group_check=True,
                )
                if t >= 1:
                    # work placed inside the broadcast-flight gap: a junk
                    # matmul after the c-update keeps HAM warm; the step's
                    # phase-1 group runs after the h-cast.
                    gr = p1_group_at_step(t - 1)
                    if GAP_JUNK:
                        te.wait_ge(dvec_sem, t)
                        junk_mm()
                    if gr is not None or GAP_JUNK:
                        te.wait_ge(dcast_sem, t)
                    if gr is not None:
                        n, j = gr
                        if n >= P1_PRO_N_ and j == 0:
                            te.wait_ge(p1d_sems[n % N_STAG], 64 * (n // N_STAG + 1))
                        p1_mms(n, j)
                    elif GAP_JUNK:
                        junk_mm()
                    # PE fill while the broadcast is in flight (HAM warmth)
                    for _ in range(4 if gr is not None else 16):
                        fill_mm()
                    te.wait_ge(rsems[t % 2], 16 * ((t + 1) // 2))
                for k in range(8):
                    for j in range(4):
                        te.matmul(
                            ps[t % 2][:, 32 * j : 32 * j + 32],
                            whT[:, (k * 4 + j) * 128 : (k * 4 + j) * 128 + 128],
                            hT[t % 2][:, 32 * k : 32 * k + 32],
                            start=False,
                            stop=(k == 7),
                            skip_group_check=True,
                        ).then_maybe_inc(
                            (mm_sem, 1) if (k == 7 and j == 3) else None
                        )

        # ---------------- SCALAR engine (ACT) ----------------
        @blk.scalar
        def _(a):
            a.wait_ge(ld_sem, 80)
            for t in range(T_):
                a.wait_ge(mm_sem, t + 1)
                if t >= 2:
                    a.wait_ge(dveh_sem, t - 1)  # sig/g/tc bufs freed by DVE(t-2)
                a.activation(sig_sb[t % 2][:], ps[t % 2][:, 0:96], AF.Sigmoid).then_inc(
                    actsig_sem, 1
                )
                a.activation(g_sb[t % 2][:], ps[t % 2][:, 96:128], AF.Tanh).then_inc(
                    acts_sem, 1
                )
                a.wait_ge(dvec_sem, t + 1)
                a.activation(tc_sb[t % 2][:], c_sb[:], AF.Tanh).then_inc(acttc_sem, 1)
                if CAST_ON_ACT and not DVE_DIRECT_SEND:
                    a.wait_ge(dveh_sem, t + 1)
                    if t >= 2:
                        a.wait_ge(lsems[t % 2], 16 * ((t - 2) // 2 + 1))
                    a.activation(h_send[t % 2][:], h_f32[t % 2][:], AF.Copy).then_inc(
                        dcast_sem, 1
                    )

        # ---------------- VECTOR engine (DVE) ----------------
        @blk.vector
        def _(v):
            v.wait_ge(ld_sem, 80)
            v.wait_ge(init_sem, 3)

            def p1_copy(n, j):
                v.wait_ge(p1m_sem, p1_group_index(n, j) + 1)
                v.tensor_scalar_add(xg_dst(n, j), ps1[j][:], bT[:, j : j + 1]).then_inc(
                    p1c_sem, 1
                )

            for n in range(P1_PRO_N_):
                for j in range(4):
                    p1_copy(n, j)

            for t in range(T_):
                v.wait_ge(actsig_sem, t + 1)
                v.tensor_mul(c_sb[:], sig_sb[t % 2][:, 32:64], c_sb[:])  # c *= f
                v.wait_ge(acts_sem, t + 1)
                v.tensor_mul(ig_sb[:], sig_sb[t % 2][:, 0:32], g_sb[t % 2][:])
                v.tensor_add(c_sb[:], c_sb[:], ig_sb[:]).then_inc(dvec_sem, 1)
                v.wait_ge(acttc_sem, t + 1)
                if t >= 2:
                    v.wait_ge(odma_sems[t % 2], 16 * ((t - 2) // 2 + 1))  # h_f32[t%2] freed
                if DVE_DIRECT_SEND:
                    if t >= 2:
                        v.wait_ge(lsems[t % 2], 16 * ((t - 2) // 2 + 1))
                    v.tensor_mul(
                        h_send[t % 2][:], sig_sb[t % 2][:, 64:96], tc_sb[t % 2][:]
                    ).then_inc(dcast_sem, 1)
                v.tensor_mul(
                    h_f32[t % 2][:], sig_sb[t % 2][:, 64:96], tc_sb[t % 2][:]
                ).then_inc(dveh_sem, 1)
                if not CAST_ON_ACT:
                    if t >= 2:
                        # h_send[t%2] released once bcast(t-2) finished reading
                        v.wait_ge(lsems[t % 2], 16 * ((t - 2) // 2 + 1))
                    v.tensor_copy(h_send[t % 2][:], h_f32[t % 2][:]).then_inc(
                        dcast_sem, 1
                    )
                gr = p1_group_at_step(t - 1)
                if gr is not None:
                    p1_copy(*gr)

    nc.compile()
    return nc


def _host_prep(data, Wx, Wh, b):
    """Build per-core input maps (bf16 conversion + lhsT tile layouts)."""
    bf = ml_dtypes.bfloat16
    # reference gate order is [i, f, g, o]; device block order is [i, f, o, g]
    gate_base = [0 * U, 1 * U, 3 * U, 2 * U]

    dataT = np.ascontiguousarray(np.transpose(data, (2, 1, 0)))  # [D, T, B]
    dataT = dataT.reshape(4, 128, T * B).astype(bf)

    ident = np.eye(128, dtype=np.float32).astype(bf)

    Wh_bf = Wh.astype(bf)
    Wx_bf = Wx.astype(bf)

    in_maps = []
    for c in range(N_CORES):
        u0 = c * UC
        whT = np.empty((128, 8 * 4 * 128), dtype=bf)
        for k in range(8):
            for j in range(4):
                whT[:, (k * 4 + j) * 128 : (k * 4 + j) * 128 + 128] = Wh_bf[
                    128 * k : 128 * k + 128, gate_base[j] + u0 : gate_base[j] + u0 + UC
                ]
        wxT = np.empty((128, 4 * 4 * 128), dtype=bf)
        for k in range(4):
            for j in range(4):
                wxT[:, (k * 4 + j) * 128 : (k * 4 + j) * 128 + 128] = Wx_bf[
                    128 * k : 128 * k + 128, gate_base[j] + u0 : gate_base[j] + u0 + UC
                ]
        bT = np.stack(
            [b[gate_base[j] + u0 : gate_base[j] + u0 + UC] for j in range(4)], axis=1
        ).astype(np.float32)
        in_maps.append(
            {
                "whT": whT,
                "wxT": wxT,
                "ident": ident,
                "bT": np.ascontiguousarray(bT),
                "cfg": np.array([[c]], dtype=np.uint32),
                "dataT": dataT,
            }
        )
    return in_maps


def kernel(data, Wx, Wh, b):
    global _compiled, LAST_EXEC_NS, TRACE_DIR
    from concourse.bass_utils import run_bass_kernel_spmd

    if _compiled is None:
        _compiled = _build()
    in_maps = _host_prep(
        np.asarray(data), np.asarray(Wx), np.asarray(Wh), np.asarray(b)
    )
    kw = {}
    if TRACE:
        import tempfile
        import types

        if "antenv.axon_hooks" not in sys.modules:
            m = types.ModuleType("antenv.axon_hooks")
            m._hook = None

            def _set(hook):
                m._hook = hook

            def _get():
                if m._hook is None:
                    try:
                        from trn_agent_boot.trn_boot import _ntff_profile_via_ctypes

                        m._hook = _ntff_profile_via_ctypes("/opt/axon/libaxon_pjrt.so")
                    except Exception:
                        m._hook = None
                return m._hook

            m.set_axon_ntff_profile_hook = _set
            m.get_axon_ntff_profile_hook = _get
            sys.modules["antenv.axon_hooks"] = m
            import antenv

            antenv.axon_hooks = m

        TRACE_DIR = tempfile.mkdtemp(prefix="lstm_trace_")
        kw = dict(trace=True, tmpdir=TRACE_DIR)
    res = run_bass_kernel_spmd(
        _compiled, in_maps, core_ids=list(range(N_CORES)), **kw
    )
    LAST_EXEC_NS = res.exec_time_ns
    out = np.empty((B, T, U), dtype=np.float32)
    for c in range(N_CORES):
        out[:, :, c * UC : (c + 1) * UC] = np.transpose(
            res.results[c]["out_hT"].astype(np.float32), (2, 0, 1)
        )
    return out


if __name__ == "__main__":
    ins = dict(np.load("/root/problem/inputs.npz"))
    out = kernel(**ins)
    exp = np.load("/root/problem/expected.npy")
    err = np.abs(out - exp)
    print("abs max err:", err.max(), "rel:", err.max() / np.abs(exp).max())

